# revision 26
# baseline (speedup 1.0000x reference)
"""Trainium2 Bass kernel for a Mistral-style cross-attention transformer block.

Sharding (8 NeuronCores, tensor-parallel, zero on-device collectives):
  Launch 1 (attention): cores grouped by batch (4 cores/batch); each core
    computes 8 q-heads / 2 kv-heads of cross-attention for its batch and a
    partial O-projection. Host sums the 4 partials per batch + residual.
  Launch 2 (MLP): Megatron split of the SwiGLU intermediate dim (2048 per
    core); each core emits a partial down-projection. Host sums + residual.

Perf design (vs the first working version):
  - projection/MLP matmuls use dual-fp8 "hi+lo" DoubleRow: W ~ Whi+Wlo and
    x ~ xhi+xlo in e4m3; Whi@xhi pairs 2 k-tiles per DR instruction and one
    DR instruction per k-tile packs the (Whi@xlo + Wlo@xhi) cross terms.
    0.75x of bf16 PE cycles (DR costs 0.5 cyc/row) at ~0.1% matmul error.
    Weights are split host-side (scaled x256/x4 to dodge e4m3 subnormals
    AND the ~240 e4m3 inf threshold); activation splits ride ACT (hi cast)
    + DVE (lo subtract). Scales cancel in exp / output copies.
  - attention core (scores / softmax / PV / denominator) stays bf16:
    K=128 contraction cannot DoubleRow at full width and fp8 probs fail
    the error budget; statistics/softmax in f32 via PSUM.
  - scores computed TRANSPOSED (S^T[k,q]) so the mask is a free per-partition
    activation bias, softmax probs feed the PV matmul with no transposes, and
    normalization is applied to ctx via one K=1 broadcast matmul + one mult.
  - LN scale folded into q/gate/up weights on host; LN bias enters via K=1
    matmuls into the accumulating PSUM (generically correct, zero here).
  - weights host-pre-tiled so every DMA is a few large descriptors; DMA
    instruction count cut ~20x (HWDGE/SP.SEQ serialization was a bottleneck).
"""
import numpy as np
import ml_dtypes

import concourse.mybir as mybir
import concourse.tile as tile
from concourse import bacc
from concourse.bass_utils import run_bass_kernel_spmd
from concourse.masks import make_identity

B, QL, KVL, D = 2, 1024, 2048, 4096
NH, NKV, HD = 32, 8, 128
INNER = 4 * D
EPS = 1e-5
THETA = 10000.0
NCORES = 8
P = 128
F32 = mybir.dt.float32
BF16 = mybir.dt.bfloat16
AX = mybir.AxisListType.X
ALU = mybir.AluOpType
ACTF = mybir.ActivationFunctionType
BFNP = ml_dtypes.bfloat16

H_LOC = NH // (NCORES // B)      # 8 q heads per core
KV_LOC = NKV // (NCORES // B)    # 2 kv heads per core
J_LOC = INNER // NCORES          # 2048 intermediate dims per core
DK = D // P                      # 32 k-tiles over hidden dim
TQ = QL // P                     # 8 query tiles
TK = KVL // P                    # 16 key tiles
JK = J_LOC // P                  # 16 j-tiles per core


def _ln_stats(nc, stat, sq_pool, x_t, eps_ap, width):
    """rstd [P,1], nmur [P,1] = -mu*rstd from x_t [P,width].

    Both reductions ride the ACT accumulator so the DVE stays free for
    the transpose-split traffic that shares the same program phase."""
    s1 = stat.tile([P, 1], F32, name="s1")
    s2 = stat.tile([P, 1], F32, name="s2")
    sq = sq_pool.tile([P, width], BF16, name="sq")
    nc.scalar.activation(sq[:], x_t[:], ACTF.Identity, accum_out=s1[:])
    nc.scalar.activation(sq[:], x_t[:], ACTF.Square, accum_out=s2[:])
    mu = stat.tile([P, 1], F32, name="mu")
    m2 = stat.tile([P, 1], F32, name="m2")
    nc.vector.tensor_scalar_mul(mu[:], s1[:], 1.0 / width)
    nc.vector.tensor_scalar_mul(m2[:], s2[:], 1.0 / width)
    musq = stat.tile([P, 1], F32, name="musq")
    var = stat.tile([P, 1], F32, name="var")
    nc.vector.tensor_tensor(out=musq[:], in0=mu[:], in1=mu[:], op=ALU.mult)
    nc.vector.tensor_tensor(out=var[:], in0=m2[:], in1=musq[:], op=ALU.subtract)
    std = stat.tile([P, 1], F32, name="std")
    nc.scalar.activation(std[:], var[:], ACTF.Sqrt, bias=eps_ap)
    rstd = stat.tile([P, 1], F32, name="rstd")
    nc.vector.reciprocal(rstd[:], std[:])
    nmur = stat.tile([P, 1], F32, name="nmur")
    nc.vector.tensor_scalar(
        out=nmur[:], in0=mu[:], scalar1=rstd[:], scalar2=-1.0,
        op0=ALU.mult, op1=ALU.mult,
    )
    return rstd, nmur


def _ln_transpose_block(nc, stat, sq_pool, tmp_pool, ps_pool, ident_bf,
                        eps_ap, x_t, dst, tok_slice):
    """LN-normalize x_t [P, D] (no scale/bias) -> bf16, transpose into
    dst[:, k, tok_slice] for all DK k-tiles."""
    rstd, nmur = _ln_stats(nc, stat, sq_pool, x_t, eps_ap, D)
    tmp = tmp_pool.tile([P, D], BF16, name="tmp")
    nc.vector.tensor_scalar(
        out=tmp[:], in0=x_t[:], scalar1=rstd[:], scalar2=nmur[:],
        op0=ALU.mult, op1=ALU.add,
    )
    for kq in range(DK // 4):
        ps_t = ps_pool.tile([P, 512], BF16, name="ps_t")
        for i in range(4):
            k = kq * 4 + i
            nc.tensor.transpose(ps_t[:, i * P:(i + 1) * P],
                                tmp[:, k * P:(k + 1) * P], ident_bf[:])
        nc.vector.tensor_copy(dst[:, kq * 4:kq * 4 + 4, tok_slice], ps_t[:])


def _rope_from_psum(nc, scratch, psrc, cos_ap, sin_ap, out_ap, width):
    """out = psrc*cos + shiftswap(psrc)*sin ; psrc is a PSUM [128,width] AP.

    The partition-swap copies run on ACT (idle in these phases); the rot
    halves land in bf16 so the sin-mult gets the DVE 2x mode."""
    half = HD // 2
    rot = scratch.tile([P, width], BF16, name="rope_rot")
    nc.scalar.activation(rot[0:half, :], psrc[half:P, :], ACTF.Copy)
    nc.scalar.activation(rot[half:P, :], psrc[0:half, :], ACTF.Copy)
    t2 = scratch.tile([P, width], F32, name="rope_t2")
    nc.vector.tensor_tensor(out=t2[:], in0=psrc, in1=cos_ap, op=ALU.mult)
    t1 = scratch.tile([P, width], BF16, name="rope_t1")
    nc.vector.tensor_tensor(out=t1[:], in0=rot[:], in1=sin_ap, op=ALU.mult)
    nc.vector.tensor_tensor(out=out_ap, in0=t2[:], in1=t1[:], op=ALU.add)


def _ln_stats_bn(nc, stat, x_t, eps_ap, width):
    """rstd, nmur via DVE bn_stats/bn_aggr: one pass over x, no ACT work.

    Used in L2 where the ACT double-pass gated the block preamble; L1's
    attention phase C keeps the ACT version since its DVE is the tighter
    engine there."""
    ns = width // 512
    stats = stat.tile([P, ns, 6], F32, name="bnst")
    for i in range(ns):
        nc.vector.bn_stats(out=stats[:, i, :],
                           in_=x_t[:, i * 512:(i + 1) * 512])
    mv = stat.tile([P, 2], F32, name="bnmv")
    nc.vector.bn_aggr(out=mv[:], in_=stats[:])
    std = stat.tile([P, 1], F32, name="std")
    nc.scalar.activation(std[:], mv[:, 1:2], ACTF.Sqrt, bias=eps_ap)
    rstd = stat.tile([P, 1], F32, name="rstd")
    nc.vector.reciprocal(rstd[:], std[:])
    nmur = stat.tile([P, 1], F32, name="nmur")
    nc.vector.tensor_scalar(
        out=nmur[:], in0=mv[:, 0:1], scalar1=rstd[:], scalar2=-1.0,
        op0=ALU.mult, op1=ALU.mult,
    )
    return rstd, nmur


SQ = 256.0   # q-weight fp8 scale
SK = 256.0   # k-weight fp8 scale
SV = 16.0    # v-weight fp8 scale
SO = 256.0   # o-weight fp8 scale


def build_l1():
    """Cross-attention with dual-fp8 Q/K/V/O projections (see build_l2 doc).

    Scores / PV / softmax stay bf16: their contraction depth (128) cannot
    use DoubleRow at full partition width, and fp8 probs fail the error
    budget. Weight scales (SQ*SK) cancel inside exp via the ACT scale;
    (SV*SO) cancels in the output copy.
    """
    nc = bacc.Bacc("TRN2", target_bir_lowering=False, debug=False)

    x = nc.dram_tensor("x", (QL, D), BF16, kind="ExternalInput")
    encc = nc.dram_tensor("encc", (8 * P, DK * 2 * 256), FP8,
                          kind="ExternalInput")
    qwc = nc.dram_tensor("qwc", (H_LOC * P, DK * 2 * HD), FP8,
                         kind="ExternalInput")
    kwc = nc.dram_tensor("kwc", (P, KV_LOC * DK * 2 * HD), FP8,
                         kind="ExternalInput")
    vwc = nc.dram_tensor("vwc", (P, KV_LOC * DK * 2 * HD), FP8,
                         kind="ExternalInput")
    owc = nc.dram_tensor("owc", (8 * P, H_LOC * 2 * 512), FP8,
                         kind="ExternalInput")
    qb = nc.dram_tensor("qb", (1, H_LOC * HD), BF16, kind="ExternalInput")
    cosq = nc.dram_tensor("cosq", (HD, QL), F32, kind="ExternalInput")
    sinq = nc.dram_tensor("sinq", (HD, QL), F32, kind="ExternalInput")
    cosk = nc.dram_tensor("cosk", (HD, KVL), BF16, kind="ExternalInput")
    sink = nc.dram_tensor("sink", (HD, KVL), BF16, kind="ExternalInput")
    maskt = nc.dram_tensor("maskt", (P, TK), F32, kind="ExternalInput")
    attn_part = nc.dram_tensor("attn_part", (QL, D), F32, kind="ExternalOutput")

    with tile.TileContext(nc) as tc:
        with (
            tc.tile_pool(name="const", bufs=1) as const,
            tc.tile_pool(name="stat", bufs=4) as stat,
            tc.tile_pool(name="res", bufs=1) as res,
        ):
            id32 = const.tile([P, P], F32, name="id32")
            make_identity(nc, id32[:])
            ident_bf = const.tile([P, P], BF16, name="ident_bf")
            nc.vector.tensor_copy(ident_bf[:], id32[:])
            eps_t = const.tile([P, 1], F32, name="eps_t")
            nc.vector.memset(eps_t[:], EPS)
            eps_ap = eps_t[:]
            ones_col = const.tile([P, 1], BF16, name="ones_col")
            nc.vector.memset(ones_col[:], 1.0)
            ones1 = const.tile([1, P], BF16, name="ones1")
            nc.vector.memset(ones1[:], 1.0)
            ones_row = const.tile([1, 512], BF16, name="ones_row")
            nc.vector.memset(ones_row[:], 1.0)
            qb_sb = const.tile([1, H_LOC * HD], BF16, name="qb_sb")
            mraw = const.tile([P, TK], F32, name="mraw")
            maskb = const.tile([P, TK], F32, name="maskb")

            # hT slots: [:, kt, 0, :]=hi fp8, [:, kt, 1, :]=lo
            hT = res.tile([P, DK, 2, QL], FP8, name="hT")
            kT = res.tile([P, KV_LOC, KVL], BF16, name="kT")
            v_nat = res.tile([P, KV_LOC, TK, HD], BF16, name="v_nat")

            # ---- Phases C+A interleaved: 8 chunks of 256 keys + LN1 ----
            with (
                tc.tile_pool(name="c_w", bufs=1) as cw,
                tc.tile_pool(name="c_enc", bufs=2) as cenc,
                tc.tile_pool(name="c_cs", bufs=1) as ccs,
                tc.tile_pool(name="c_s", bufs=2) as cs,
                tc.tile_pool(name="a_x", bufs=2) as ax,
                tc.tile_pool(name="a_sq", bufs=1) as asq,
                tc.tile_pool(name="a_tmp", bufs=2) as atmp,
                tc.tile_pool(name="c_ps", bufs=1, space="PSUM") as cps,
                tc.tile_pool(name="c_tps", bufs=2, space="PSUM") as ctps,
                tc.tile_pool(name="a_ps", bufs=2, space="PSUM") as aps,
            ):
                kw_sb = cw.tile([P, KV_LOC, DK, 2, HD], FP8, name="kw_sb")
                vw_sb = cw.tile([P, KV_LOC, DK, 2, HD], FP8, name="vw_sb")
                nc.sync.dma_start(out=kw_sb[:], in_=kwc[:, :])
                nc.sync.dma_start(out=vw_sb[:], in_=vwc[:, :])
                cosk_sb = ccs.tile([HD, KVL], BF16, name="cosk")
                sink_sb = ccs.tile([HD, KVL], BF16, name="sink")
                for c8 in range(8):
                    ck = slice(c8 * 256, (c8 + 1) * 256)
                    enc_t = cenc.tile([P, DK, 2, 256], FP8, name="enc_t")
                    nc.sync.dma_start(out=enc_t[:],
                                      in_=encc[c8 * P:(c8 + 1) * P, :])
                    # LN1 chain for query tile c8 (DVE/ACT; hidden by MMs)
                    x_t = ax.tile([P, D], BF16, name="x_t")
                    nc.sync.dma_start(out=x_t[:],
                                      in_=x[c8 * P:(c8 + 1) * P, :])
                    rstd, nmur = _ln_stats(nc, stat, asq, x_t, eps_ap, D)
                    tmp = atmp.tile([P, D], BF16, name="tmp")
                    nc.vector.tensor_scalar(
                        out=tmp[:], in0=x_t[:], scalar1=rstd[:],
                        scalar2=nmur[:], op0=ALU.mult, op1=ALU.add,
                    )
                    if c8 == 0:
                        # deferred small DMAs: behind the critical kw/vw/enc
                        # transfers, still ahead of their consumers
                        nc.sync.dma_start(out=cosk_sb[:], in_=cosk[:, :])
                        nc.sync.dma_start(out=sink_sb[:], in_=sink[:, :])
                        nc.sync.dma_start(out=qb_sb[:], in_=qb[:, :])
                        nc.sync.dma_start(out=mraw[:], in_=maskt[:, :])
                        nc.vector.tensor_scalar(out=maskb[:], in0=mraw[:],
                                                scalar1=0.0, scalar2=None,
                                                op0=ALU.not_equal)
                    # K/V chunk matmuls (dual-fp8 3-term)
                    pk = [cps.tile([P, 256], F32, name=f"pk{i}")
                          for i in range(KV_LOC)]
                    pv = [cps.tile([P, 256], F32, name=f"pv{i}")
                          for i in range(KV_LOC)]
                    for kv in range(KV_LOC):
                        for kp in range(DK // 2):
                            ks = slice(2 * kp, 2 * kp + 2)
                            nc.tensor.matmul(pk[kv][:],
                                             kw_sb[:, kv, ks, 1, :],
                                             enc_t[:, ks, 0, :],
                                             start=(kp == 0), stop=False,
                                             perf_mode=DRM)
                            nc.tensor.matmul(pv[kv][:],
                                             vw_sb[:, kv, ks, 1, :],
                                             enc_t[:, ks, 0, :],
                                             start=(kp == 0), stop=False,
                                             perf_mode=DRM)
                        for kt in range(DK):
                            nc.tensor.matmul(pk[kv][:],
                                             kw_sb[:, kv, kt, :, :],
                                             enc_t[:, kt, :, :],
                                             start=False, stop=(kt == DK - 1),
                                             perf_mode=DRM)
                            nc.tensor.matmul(pv[kv][:],
                                             vw_sb[:, kv, kt, :, :],
                                             enc_t[:, kt, :, :],
                                             start=False, stop=(kt == DK - 1),
                                             perf_mode=DRM)
                    for kv in range(KV_LOC):
                        _rope_from_psum(nc, cs, pk[kv][:], cosk_sb[:, ck],
                                        sink_sb[:, ck], kT[:, kv, ck], 256)
                        vt_sb = cs.tile([P, 256], BF16, name="vt_sb")
                        nc.vector.tensor_copy(vt_sb[:], pv[kv][:])
                        ps_v = ctps.tile([P, 256], BF16, name="ps_v")
                        for i in range(2):
                            nc.tensor.transpose(ps_v[:, i * P:(i + 1) * P],
                                                vt_sb[:, i * P:(i + 1) * P],
                                                ident_bf[:])
                        nc.vector.tensor_copy(
                            v_nat[:, kv, c8 * 2:c8 * 2 + 2, :], ps_v[:])
                    # transposes of LN tile c8 -> hT hi/lo slots
                    tok = slice(c8 * P, (c8 + 1) * P)
                    for kq in range(DK // 4):
                        ps_t = aps.tile([P, 512], BF16, name="ps_t")
                        for i in range(4):
                            k = kq * 4 + i
                            nc.tensor.transpose(
                                ps_t[:, i * P:(i + 1) * P],
                                tmp[:, k * P:(k + 1) * P], ident_bf[:])
                        hi_ap = hT[:, kq * 4:kq * 4 + 4, 0, tok]
                        nc.scalar.activation(hi_ap, ps_t[:], ACTF.Copy)
                        nc.vector.tensor_tensor(
                            out=hT[:, kq * 4:kq * 4 + 4, 1, tok],
                            in0=ps_t[:], in1=hi_ap, op=ALU.subtract)

            with tc.tile_pool(name="res2", bufs=1) as res2:
                qT = res2.tile([P, H_LOC, QL], BF16, name="qT")
                ctx = res2.tile([P, H_LOC, 2, QL], FP8, name="ctx")
                # ---- Phase B: Q projection (dual-fp8) + bias + RoPE ----
                with (
                    tc.tile_pool(name="b_w", bufs=2) as bw,
                    tc.tile_pool(name="b_cs", bufs=1) as bcs,
                    tc.tile_pool(name="b_s", bufs=2) as bs,
                    tc.tile_pool(name="b_ps", bufs=2, space="PSUM") as bps,
                ):
                    cosq_sb = bcs.tile([HD, QL], F32, name="cosq")
                    sinq_sb = bcs.tile([HD, QL], F32, name="sinq")
                    nc.sync.dma_start(out=cosq_sb[:], in_=cosq[:, :])
                    nc.sync.dma_start(out=sinq_sb[:], in_=sinq[:, :])
                    for h in range(H_LOC):
                        qw_h = bw.tile([P, DK, 2, HD], FP8, name="qw_h")
                        nc.sync.dma_start(out=qw_h[:],
                                          in_=qwc[h * P:(h + 1) * P, :])
                        pq = [bps.tile([P, 512], F32, name=f"pq{i}")
                              for i in range(2)]
                        for kp in range(DK // 2):
                            ks = slice(2 * kp, 2 * kp + 2)
                            for th in range(2):
                                ts = slice(th * 512, (th + 1) * 512)
                                nc.tensor.matmul(
                                    pq[th][:], qw_h[:, ks, 1, :],
                                    hT[:, ks, 0, ts],
                                    start=(kp == 0), stop=False,
                                    perf_mode=DRM)
                        for kt in range(DK):
                            for th in range(2):
                                ts = slice(th * 512, (th + 1) * 512)
                                nc.tensor.matmul(
                                    pq[th][:], qw_h[:, kt, :, :],
                                    hT[:, kt, :, ts],
                                    start=False, stop=False, perf_mode=DRM)
                        for th in range(2):
                            nc.tensor.matmul(
                                pq[th][:], qb_sb[0:1, h * HD:(h + 1) * HD],
                                ones_row[0:1, :], start=False, stop=True)
                            sl = slice(th * 512, (th + 1) * 512)
                            _rope_from_psum(nc, bs, pq[th][:], cosq_sb[:, sl],
                                            sinq_sb[:, sl], qT[:, h, sl], 512)

                # ---- Phases D+E: attention (bf16 core) with the partial
                # O-projection of each query half interleaved so its DR
                # matmuls fill the PE while the other half's softmax exps
                # occupy the ACT engine. ----
                with (
                    tc.tile_pool(name="d_es", bufs=2) as des,
                    tc.tile_pool(name="d_s", bufs=2) as dsc,
                    tc.tile_pool(name="e_w", bufs=2) as ew,
                    tc.tile_pool(name="e_s", bufs=3) as es_pool,
                    tc.tile_pool(name="d_sps", bufs=3, space="PSUM") as dsps,
                    tc.tile_pool(name="d_aps", bufs=1, space="PSUM") as daps,
                    tc.tile_pool(name="d_cps", bufs=2, space="PSUM") as dcps,
                    tc.tile_pool(name="e_ps", bufs=2, space="PSUM") as eps_pool,
                ):
                    for qc in range(2):
                        qsl = slice(qc * 512, (qc + 1) * 512)
                        for h in range(H_LOC):
                            kv = h // (H_LOC // KV_LOC)
                            es = des.tile([P, TK, 512], BF16, name="es")
                            # ps_sum and ps_rb share one bank: ps_rb's write
                            # comes after the reciprocal has consumed ps_sum
                            ps_aux = daps.tile([P, 512], F32, name="ps_aux")
                            ps_sum = ps_aux[0:1, :]
                            ps_ctx = dcps.tile([P, 512], F32, name="ps_ctx")

                            def flush(k):
                                nc.tensor.matmul(ps_sum, ones_col[:],
                                                 es[:, k, :], start=(k == 0),
                                                 stop=(k == TK - 1))
                                nc.tensor.matmul(ps_ctx[:], v_nat[:, kv, k, :],
                                                 es[:, k, :], start=(k == 0),
                                                 stop=(k == TK - 1))

                            for kt in range(TK):
                                ps_s = dsps.tile([P, 512], F32, name="ps_s")
                                nc.tensor.matmul(ps_s[:],
                                                 kT[:, kv, kt * P:(kt + 1) * P],
                                                 qT[:, h, qsl], start=True,
                                                 stop=True)
                                nc.scalar.activation(es[:, kt, :], ps_s[:],
                                                     ACTF.Exp,
                                                     scale=1.0 / (SQ * SK),
                                                     bias=maskb[:, kt:kt + 1])
                                if kt >= 3:
                                    flush(kt - 3)
                            flush(TK - 3)
                            flush(TK - 2)
                            flush(TK - 1)
                            rr = dsc.tile([1, 512], BF16, name="rr")
                            with nc.allow_low_precision(reason="softmax recip bf16"):
                                nc.vector.reciprocal(rr[:], ps_sum)
                            nc.tensor.matmul(ps_aux[:], ones1[:], rr[:],
                                             start=True, stop=True)
                            rb = dsc.tile([P, 512], F32, name="rb")
                            nc.vector.tensor_copy(rb[:], ps_aux[:])
                            t_sb = dsc.tile([P, 512], BF16, name="t_sb")
                            nc.vector.tensor_tensor(out=t_sb[:], in0=ps_ctx[:],
                                                    in1=rb[:], op=ALU.mult)
                            hi_ap = ctx[:, h, 0, qsl]
                            nc.scalar.activation(hi_ap, t_sb[:], ACTF.Copy)
                            nc.vector.tensor_tensor(out=ctx[:, h, 1, qsl],
                                                    in0=t_sb[:], in1=hi_ap,
                                                    op=ALU.subtract)
                    # O projection after both query halves (dual-fp8):
                    # its DR matmuls follow D's PE stream while the last
                    # head's softmax tail drains on ACT/DVE.
                    if qc == 1:
                        for do in range(8):
                            ow_do = ew.tile([P, H_LOC, 2, 512], FP8,
                                            name="ow_do")
                            nc.sync.dma_start(out=ow_do[:],
                                              in_=owc[do * P:(do + 1) * P, :])
                            for qt in range(TQ):
                                qtl = slice(qt * P, (qt + 1) * P)
                                ps_o = eps_pool.tile([P, 512], F32, name="ps_o")
                                for hp in range(H_LOC // 2):
                                    hs = slice(2 * hp, 2 * hp + 2)
                                    nc.tensor.matmul(ps_o[:],
                                                     ctx[:, hs, 0, qtl],
                                                     ow_do[:, hs, 1, :],
                                                     start=(hp == 0),
                                                     stop=False, perf_mode=DRM)
                                for h in range(H_LOC):
                                    nc.tensor.matmul(ps_o[:],
                                                     ctx[:, h, :, qtl],
                                                     ow_do[:, h, :, :],
                                                     start=False,
                                                     stop=(h == H_LOC - 1),
                                                     perf_mode=DRM)
                                o_sb = es_pool.tile([P, 512], F32, name="o_sb")
                                nc.vector.tensor_scalar(
                                    out=o_sb[:], in0=ps_o[:],
                                    scalar1=1.0 / (SV * SO), scalar2=None,
                                    op0=ALU.mult)
                                nc.sync.dma_start(
                                    out=attn_part[qt * P:(qt + 1) * P,
                                                  do * 512:(do + 1) * 512],
                                    in_=o_sb[:])
    nc.compile()
    return nc


ACT_GATE = ACTF.Silu   # swapped to Sigmoid by the CoreSim dev harness only
SG = 256.0   # gate-weight fp8 scale
SU = 4.0     # up-weight fp8 scale (keeps SU*ff_pre well under the e4m3 inf threshold ~240)
SD = 256.0   # down-weight fp8 scale
FP8 = mybir.dt.float8e4
DRM = mybir.MatmulPerfMode.DoubleRow


def build_l2():
    """SwiGLU MLP with dual-fp8 ("hi+lo") DoubleRow matmuls.

    Each bf16 matmul W@x is replaced by Whi@xhi + (Whi@xlo + Wlo@xhi) where
    Whi/Wlo and xhi/xlo are e4m3 splits. The hi term pairs 2 k-tiles per
    DoubleRow instruction; the cross term packs (hi,lo)x(lo,hi) slots of one
    k-tile per instruction. 3 DR instructions / 2 k-tiles = 0.75x bf16 PE
    cycles with ~0.1% matmul error (better than bf16). Weights are split on
    host (scaled by SG/SU/SD to escape e4m3 subnormals); activations are
    split on-device (2 DVE ops per [128,512] tile).
    """
    nc = bacc.Bacc("TRN2", target_bir_lowering=False, debug=False)

    h1b = nc.dram_tensor("h1b", (B * QL, D), BF16, kind="ExternalInput")
    gwc = nc.dram_tensor("gwc", (J_LOC, 2 * D), FP8, kind="ExternalInput")
    uwc = nc.dram_tensor("uwc", (J_LOC, 2 * D), FP8, kind="ExternalInput")
    dwc = nc.dram_tensor("dwc", (P, 8 * JK * 2 * 512), FP8, kind="ExternalInput")
    gb = nc.dram_tensor("gb", (P, JK), F32, kind="ExternalInput")
    ub = nc.dram_tensor("ub", (P, JK), F32, kind="ExternalInput")
    ffp = nc.dram_tensor("ffp", (B * QL, D), F32, kind="ExternalOutput")

    NBLK, TB, TPB = 4, 512, 4   # token blocks; LN(b+1) overlaps matmuls(b)

    with tile.TileContext(nc) as tc:
        with (
            tc.tile_pool(name="const", bufs=1) as const,
            tc.tile_pool(name="stat", bufs=4) as stat,
            tc.tile_pool(name="h2", bufs=2) as h2p,
            tc.tile_pool(name="ffb", bufs=1) as ffpool,
            tc.tile_pool(name="l_x", bufs=2) as lx,
            tc.tile_pool(name="l_sq", bufs=1) as lsq,
            tc.tile_pool(name="l_tmp", bufs=1) as ltmp,
            tc.tile_pool(name="g_w", bufs=2) as gwp,
            tc.tile_pool(name="g_s", bufs=2) as gs,
            tc.tile_pool(name="d_w", bufs=2) as dwp,
            tc.tile_pool(name="d_o", bufs=2) as dop,
            tc.tile_pool(name="l_ps", bufs=2, space="PSUM") as lps,
            tc.tile_pool(name="g_ps", bufs=2, space="PSUM") as gps,
            tc.tile_pool(name="d_ps", bufs=2, space="PSUM") as dps,
        ):
            id32 = const.tile([P, P], F32, name="id32")
            make_identity(nc, id32[:])
            ident_bf = const.tile([P, P], BF16, name="ident_bf")
            nc.vector.tensor_copy(ident_bf[:], id32[:])
            eps_t = const.tile([P, 1], F32, name="eps_t")
            nc.vector.memset(eps_t[:], EPS)
            eps_ap = eps_t[:]
            gb_sb = const.tile([P, JK], F32, name="gb_sb")
            nc.sync.dma_start(out=gb_sb[:], in_=gb[:, :])
            ub_sb = const.tile([P, JK], F32, name="ub_sb")
            nc.sync.dma_start(out=ub_sb[:], in_=ub[:, :])

            def emit_ln_block(blk, h2T):
                """LN + transpose + hi/lo split for the 4 token tiles of blk.
                Emitted between gate(b) and down(b) of the PREVIOUS block so
                the DVE/ACT burst lands in the down window where both engines
                idle, instead of stalling gate(b+1)."""
                for ti in range(TPB):
                    tt = blk * TPB + ti
                    x_t = lx.tile([P, D], BF16, name="x_t")
                    nc.sync.dma_start(out=x_t[:],
                                      in_=h1b[tt * P:(tt + 1) * P, :])
                    rstd, nmur = _ln_stats(nc, stat, lsq, x_t, eps_ap, D)
                    tmp = ltmp.tile([P, D], BF16, name="tmp")
                    nc.vector.tensor_scalar(
                        out=tmp[:], in0=x_t[:], scalar1=rstd[:],
                        scalar2=nmur[:], op0=ALU.mult, op1=ALU.add,
                    )
                    tok = slice(ti * P, (ti + 1) * P)
                    for kq in range(DK // 4):
                        ps_t = lps.tile([P, 512], BF16, name="ps_t")
                        for i in range(4):
                            k = kq * 4 + i
                            nc.tensor.transpose(ps_t[:, i * P:(i + 1) * P],
                                                tmp[:, k * P:(k + 1) * P],
                                                ident_bf[:])
                        hi_ap = h2T[:, kq * 4:kq * 4 + 4, 0, tok]
                        nc.scalar.activation(hi_ap, ps_t[:], ACTF.Copy)
                        nc.vector.tensor_tensor(
                            out=h2T[:, kq * 4:kq * 4 + 4, 1, tok],
                            in0=ps_t[:], in1=hi_ap, op=ALU.subtract)

            # h2T slots: [:, kt, 0, :] = hi(fp8), [:, kt, 1, :] = lo
            h2T = h2p.tile([P, DK, 2, TB], FP8, name="h2T")
            emit_ln_block(0, h2T)
            for blk in range(NBLK):
                ff = ffpool.tile([P, JK, 2, TB], FP8, name="ff")
                for j in range(JK):
                    gwcj = gwp.tile([P, DK, 2, P], FP8, name="gwcj")
                    uwcj = gwp.tile([P, DK, 2, P], FP8, name="uwcj")
                    nc.sync.dma_start(out=gwcj[:], in_=gwc[j * P:(j + 1) * P, :])
                    nc.sync.dma_start(out=uwcj[:], in_=uwc[j * P:(j + 1) * P, :])
                    pg = gps.tile([P, TB], F32, name="pg")
                    pu = gps.tile([P, TB], F32, name="pu")
                    for kp in range(DK // 2):
                        ks = slice(2 * kp, 2 * kp + 2)
                        nc.tensor.matmul(pg[:], gwcj[:, ks, 1, :],
                                         h2T[:, ks, 0, :], start=(kp == 0),
                                         stop=False, perf_mode=DRM)
                        nc.tensor.matmul(pu[:], uwcj[:, ks, 1, :],
                                         h2T[:, ks, 0, :], start=(kp == 0),
                                         stop=False, perf_mode=DRM)
                    for kt in range(DK):
                        nc.tensor.matmul(pg[:], gwcj[:, kt, :, :],
                                         h2T[:, kt, :, :], start=False,
                                         stop=(kt == DK - 1), perf_mode=DRM)
                        nc.tensor.matmul(pu[:], uwcj[:, kt, :, :],
                                         h2T[:, kt, :, :], start=False,
                                         stop=(kt == DK - 1), perf_mode=DRM)
                    g_sb = gs.tile([P, TB], F32, name="g_sb")
                    nc.scalar.activation(g_sb[:], pg[:], ACT_GATE,
                                         scale=1.0 / SG,
                                         bias=gb_sb[:, j:j + 1])
                    u_sb = gs.tile([P, TB], F32, name="u_sb")
                    nc.scalar.activation(u_sb[:], pu[:], ACTF.Identity,
                                         bias=ub_sb[:, j:j + 1])
                    t_sb = gs.tile([P, TB], F32, name="t_sb")
                    nc.vector.tensor_tensor(out=t_sb[:], in0=u_sb[:],
                                            in1=g_sb[:], op=ALU.mult)
                    hi_ap = ff[:, j, 0, :]
                    nc.scalar.activation(hi_ap, t_sb[:], ACTF.Copy)
                    nc.vector.tensor_tensor(out=ff[:, j, 1, :], in0=t_sb[:],
                                            in1=hi_ap, op=ALU.subtract)
                if blk + 1 < NBLK:
                    h2T = h2p.tile([P, DK, 2, TB], FP8, name="h2T")
                    emit_ln_block(blk + 1, h2T)
                for dq in range(8):
                    dwcl = dwp.tile([P, JK, 2, 512], FP8, name="dwcl")
                    nc.sync.dma_start(
                        out=dwcl[:],
                        in_=dwc[:, dq * JK * 1024:(dq + 1) * JK * 1024])
                    for tt2 in range(TPB):
                        tok = slice(tt2 * P, (tt2 + 1) * P)
                        ps_d = dps.tile([P, 512], F32, name="ps_d")
                        for jp in range(JK // 2):
                            js = slice(2 * jp, 2 * jp + 2)
                            nc.tensor.matmul(ps_d[:], ff[:, js, 0, tok],
                                             dwcl[:, js, 1, :], start=(jp == 0),
                                             stop=False, perf_mode=DRM)
                        for jk in range(JK):
                            nc.tensor.matmul(ps_d[:], ff[:, jk, :, tok],
                                             dwcl[:, jk, :, :], start=False,
                                             stop=(jk == JK - 1),
                                             perf_mode=DRM)
                        o_sb = dop.tile([P, 512], F32, name="o_sb")
                        nc.vector.tensor_scalar(
                            out=o_sb[:], in0=ps_d[:],
                            scalar1=1.0 / (SU * SD), scalar2=None,
                            op0=ALU.mult)
                        nc.sync.dma_start(
                            out=ffp[blk * TB + tt2 * P:blk * TB + (tt2 + 1) * P,
                                    dq * 512:(dq + 1) * 512],
                            in_=o_sb[:])
    nc.compile()
    return nc


def _rope_tables(seq_len, scale):
    """cosT, sinT_signed [HD, seq_len] f32; sin rows 0:63 negated; scaled."""
    exp = (np.arange(0, HD, 2).astype(np.float32) / np.float32(HD))
    inv_freq = (np.float32(1.0) / np.power(np.float32(THETA), exp)).astype(np.float32)
    t = np.arange(seq_len, dtype=np.float32)
    freqs = np.outer(t, inv_freq).astype(np.float32)
    emb = np.concatenate([freqs, freqs], axis=-1)
    cos = np.cos(emb).astype(np.float32).T * np.float32(scale)
    sin = np.sin(emb).astype(np.float32).T * np.float32(scale)
    sin_signed = sin.copy()
    sin_signed[: HD // 2] = -sin_signed[: HD // 2]
    return np.ascontiguousarray(cos), np.ascontiguousarray(sin_signed)


_CACHE = {}


def _get(name, builder):
    if name not in _CACHE:
        _CACHE[name] = builder()
    return _CACHE[name]


def _bf(a):
    return np.ascontiguousarray(np.asarray(a).astype(BFNP))


E4NP = ml_dtypes.float8_e4m3


def _split8(a):
    """hi = e4m3(a), lo = e4m3(a - hi); returns (hi, lo) e4m3 arrays."""
    hi = np.asarray(a, np.float32).astype(E4NP)
    lo = (np.asarray(a, np.float32) - hi.astype(np.float32)).astype(E4NP)
    return hi, lo


def kernel(hidden_states, encoder_hidden_states, encoder_attention_mask,
           ln1_w, ln1_b, q_w, k_w, v_w, o_w, ln2_w, ln2_b,
           gate_w, up_w, down_w):
    f32 = np.float32
    hs = np.asarray(hidden_states, f32)
    enc = np.asarray(encoder_hidden_states, f32)
    am = np.asarray(encoder_attention_mask, f32)
    q_w = np.asarray(q_w, f32)
    k_w = np.asarray(k_w, f32)
    v_w = np.asarray(v_w, f32)
    o_w = np.asarray(o_w, f32)
    ln1_w = np.asarray(ln1_w, f32)
    ln1_b = np.asarray(ln1_b, f32)
    ln2_w = np.asarray(ln2_w, f32)
    ln2_b = np.asarray(ln2_b, f32)
    gate_w = np.asarray(gate_w, f32)
    up_w = np.asarray(up_w, f32)
    down_w = np.asarray(down_w, f32)
    C = np.ascontiguousarray

    cosq, sinq = _rope_tables(QL, 1.0 / np.sqrt(np.float32(HD)))
    cosk, sink = _rope_tables(KVL, 1.0)

    qwf = q_w * ln1_w[None, :]          # fold LN1 scale into q projection
    qbias = (q_w @ ln1_b) * np.float32(SQ)

    def stk(tlo, thi):
        return np.stack([tlo, thi], axis=-2)

    nc1 = _get("l1", build_l1)
    in_maps = []
    for c in range(NCORES):
        b = c // (NCORES // B)
        g = c % (NCORES // B)
        hsl = slice(g * H_LOC * HD, (g + 1) * H_LOC * HD)
        kvsl = slice(g * KV_LOC * HD, (g + 1) * KV_LOC * HD)

        def tq(w):      # (H_LOC*HD, D) -> (H_LOC, P, DK, HD)
            return w.reshape(H_LOC, HD, DK, P).transpose(0, 3, 2, 1)

        def tkv(w):     # (KV_LOC*HD, D) -> (P, KV_LOC, DK, HD)
            return w.reshape(KV_LOC, HD, DK, P).transpose(3, 0, 2, 1)

        qhi, qlo = _split8(qwf[hsl] * np.float32(SQ))
        qwc_t = C(stk(tq(qlo), tq(qhi)).reshape(H_LOC * P, DK * 2 * HD))
        khi, klo = _split8(k_w[kvsl] * np.float32(SK))
        kwc_t = C(stk(tkv(klo), tkv(khi)).reshape(P, KV_LOC * DK * 2 * HD))
        vhi, vlo = _split8(v_w[kvsl] * np.float32(SV))
        vwc_t = C(stk(tkv(vlo), tkv(vhi)).reshape(P, KV_LOC * DK * 2 * HD))

        def to(w):      # (D, H_LOC*HD) -> (8, P, H_LOC, 512)
            return w.reshape(8, 512, H_LOC, P).transpose(0, 3, 2, 1)

        ohi, olo = _split8(o_w[:, hsl] * np.float32(SO))
        owc_t = C(stk(to(olo), to(ohi)).reshape(8 * P, H_LOC * 2 * 512))

        def te(w):      # (KVL, D) -> (8, P, DK, 256)
            return w.T.reshape(DK, P, 8, 256).transpose(2, 1, 0, 3)

        ehi, elo = _split8(enc[b])
        enc_t = C(stk(te(elo), te(ehi))[:, :, :, ::-1, :]
                  .reshape(8 * P, DK * 2 * 256))
        in_maps.append({
            "x": _bf(hs[b]),
            "encc": enc_t,
            "qwc": qwc_t, "kwc": kwc_t, "vwc": vwc_t, "owc": owc_t,
            "qb": _bf(qbias[hsl].reshape(1, H_LOC * HD)),
            "cosq": cosq, "sinq": sinq, "cosk": _bf(cosk), "sink": _bf(sink),
            "maskt": C(am[b].reshape(TK, P).T),
        })
    res1 = run_bass_kernel_spmd(nc1, in_maps, core_ids=list(range(NCORES)))

    h1 = hs.copy()
    for c in range(NCORES):
        b = c // (NCORES // B)
        h1[b] += res1.results[c]["attn_part"]

    nc2 = _get("l2", build_l2)
    h1_flat = h1.reshape(B * QL, D)
    h1b = _bf(h1_flat)

    def t4(w):          # (J_LOC, D) -> (jt, kp, kt, jp)
        return w.reshape(JK, P, DK, P).transpose(0, 3, 2, 1)

    in_maps2 = []
    for c in range(NCORES):
        jsl = slice(c * J_LOC, (c + 1) * J_LOC)
        gwf = gate_w[jsl] * ln2_w[None, :] * np.float32(SG)
        uwf = up_w[jsl] * ln2_w[None, :] * np.float32(SU)
        ghi, glo = _split8(gwf)
        uhi, ulo = _split8(uwf)
        gwc_t = C(np.stack([t4(glo), t4(ghi)], axis=3).reshape(J_LOC, 2 * D))
        uwc_t = C(np.stack([t4(ulo), t4(uhi)], axis=3).reshape(J_LOC, 2 * D))
        dwf = down_w[:, jsl].T * np.float32(SD)          # [J_LOC, D]
        dhi, dlo = _split8(dwf)

        def td(w):      # (J_LOC, D) -> (jp, jk, d)
            return w.reshape(JK, P, D).transpose(1, 0, 2)

        dwc_t = C(np.stack([td(dlo), td(dhi)], axis=2)
                  .reshape(P, JK, 2, 8, 512).transpose(0, 3, 1, 2, 4)
                  .reshape(P, 8 * JK * 2 * 512))
        in_maps2.append({
            "h1b": h1b,
            "gwc": gwc_t, "uwc": uwc_t, "dwc": dwc_t,
            "gb": C((gate_w[jsl] @ ln2_b).reshape(JK, P).T),
            "ub": C((up_w[jsl] @ ln2_b).reshape(JK, P).T * np.float32(SU)),
        })
    res2 = run_bass_kernel_spmd(nc2, in_maps2, core_ids=list(range(NCORES)))

    out = h1_flat.copy()
    for c in range(NCORES):
        out += res2.results[c]["ffp"].astype(f32)
    return out.reshape(B, QL, D)

